# revision 3
# baseline (speedup 1.0000x reference)
"""Trainium2 Bass kernel for nn_CIC (curve-attention GNN message passing block).

Sharding: data-parallel over batch B=16 -> 2 batches per core x 8 cores.
All weights replicated; no collectives.

Math (per batch b):
  att[n,l]   = sum_c w_att[c] * curves[c,n,l]
  S_l        = softmax(att, axis=l);  S_n = softmax(att, axis=n)
  curver_inter[c,n] = sum_l curves[c,n,l] * S_l[n,l]
  curves_intra[c,l] = sum_n curves[c,n,l] * S_n[n,l]
  a = wa @ curver_inter              [MID, n]
  b = wb @ curves_intra              [MID, l]
  AiT = wc.T @ a   [C, n];  AtT = wc.T @ b  [C, l]      (folded x_logits)
  logits_i = AiT.T @ x  [n, Np];  logits_t = AtT.T @ x  [l, Np]
  E = exp(logits); den = colsum(E)  (ones-matmul, replicated rows)
  UiT = a.T @ (wd[:, :MID] @ wn).T * s   [n, C]   (BN scale s folded)
  UtT = b.T @ (wd[:, MID:] @ wl).T * s   [l, C]
  y[c,p] = sum_n UiT[n,c] * E_i_norm[n,p] + sum_l UtT[l,c] * E_t_norm[l,p]
  out = leaky_relu(x + y + (beta - mean*s), 0.2)

Layout tricks:
  - curves cast to bf16 during the HBM load (SWDGE cast dma), then
    xbar-DMA-transposed to curvesT[p, t, c] = curves_bf16[c, 128*t + p].
  - curver_inter/curves_intra via 64 accumulated matmuls with sparse
    stationary weights built from S_l / S_n (block-diag / dual-diag).
  - att via one big DVE multiply + segmented reduce over curvesT.
  - Big matmul streams run as float32r (full PE rate at free dim >= 256).
"""

import numpy as np
import ml_dtypes

import concourse.bass as bass
import concourse.mybir as mybir
from concourse.tile import TileContext
from concourse.bass_utils import run_bass_kernel_spmd

bf16 = ml_dtypes.bfloat16
F32 = mybir.dt.float32
BF = mybir.dt.bfloat16
F32R = mybir.dt.float32r
ALU = mybir.AluOpType
AF = mybir.ActivationFunctionType
AX = mybir.AxisListType

B, C, N = 16, 256, 8192
CN, CL, MID = 128, 64, 128
NCORES = 8
NB = B // NCORES          # batches per core
NT = CN // 2              # 64 nl-tiles of 128 per batch
CHW = 512                 # points chunk width
NCH = N // CHW            # 16 chunks
BN_EPS = 1e-5


_WS_COUNTER = [0]


def _split_excess_waits(nc, max_waits=1):
    """This walrus build rejects instructions carrying more than ~1 sem-wait
    command.  Move excess waits onto same-engine NoOps inserted right before
    the offending instruction (program order on one engine preserves the
    semantics exactly)."""
    for fn in nc.m.functions:
        for blk in fn.blocks:
            insts = list(blk.instructions)
            out = []
            changed = False
            for inst in insts:
                si = inst.sync_info
                waits = list(si.on_wait) if si and si.on_wait else []
                if len(waits) > max_waits and inst.engine is not None:
                    keep = waits[:max_waits]
                    extra = waits[max_waits:]
                    for w in extra:
                        _WS_COUNTER[0] += 1
                        nop = mybir.InstNoOp(
                            name=f"I-waitsplit-{_WS_COUNTER[0]}",
                            opcode="NoOp",
                            engine=inst.engine,
                            ins=[],
                            outs=[],
                            sync_info=mybir.SyncInfo(on_wait=[w], on_update=[]),
                        )
                        out.append(nop)
                    si.on_wait = keep
                    changed = True
                out.append(inst)
            if changed:
                blk.instructions = out


def _act_recip(nc, out_ap, in_ap):
    """ACT-engine reciprocal via raw InstActivation (the bass wrapper refuses
    Reciprocal for accuracy reasons; softmax denominators tolerate it)."""
    eng = nc.scalar
    imm = lambda v: mybir.ImmediateValue(dtype=mybir.dt.float32, value=v)
    return eng.add_instruction(
        mybir.InstActivation(
            name=nc.get_next_instruction_name(),
            func=AF.Reciprocal,
            ins=[eng.lower_ap(in_ap), imm(0.0), imm(1.0), imm(0.0)],
            outs=[eng.lower_ap(out_ap)],
        )
    )


def _r(ap):
    """View an fp32 AP as float32r for full-rate PE streaming."""
    return ap.bitcast(F32R)


def _build_bass():
    nc = bass.Bass()

    x_d = nc.dram_tensor("x", [NB, C, N], F32R, kind="ExternalInput")
    cu_d = nc.dram_tensor("curves", [NB, C, N], F32, kind="ExternalInput")
    wc_d = nc.dram_tensor("wc", [MID, C], F32R, kind="ExternalInput")
    waT_d = nc.dram_tensor("waT", [C, MID], BF, kind="ExternalInput")
    wbT_d = nc.dram_tensor("wbT", [C, MID], BF, kind="ExternalInput")
    wdnT_d = nc.dram_tensor("wdnT", [MID, C], F32R, kind="ExternalInput")
    wdlT_d = nc.dram_tensor("wdlT", [MID, C], F32R, kind="ExternalInput")
    wattr_d = nc.dram_tensor("wattr", [128, C], BF, kind="ExternalInput")
    onesn_d = nc.dram_tensor("onesn", [128, 128], BF, kind="ExternalInput")
    onesl_d = nc.dram_tensor("onesl", [64, 64], BF, kind="ExternalInput")
    i2_d = nc.dram_tensor("i2", [128, 64], BF, kind="ExternalInput")
    eyef_d = nc.dram_tensor("eyef", [128, 128], F32, kind="ExternalInput")
    eyeb_d = nc.dram_tensor("eyeb", [128, 128], BF, kind="ExternalInput")
    betap_d = nc.dram_tensor("betap", [C, 1], F32, kind="ExternalInput")
    y_d = nc.dram_tensor("y", [NB, C, N], F32, kind="ExternalOutput")

    from contextlib import ExitStack

    with TileContext(nc) as tc:
        with ExitStack() as _es:
            cpool = _es.enter_context(tc.tile_pool(name="const", bufs=1))
            cnpool = _es.enter_context(tc.tile_pool(name="cnat", bufs=2))
            ctpool = _es.enter_context(tc.tile_pool(name="ctrans", bufs=1))
            prpool = _es.enter_context(tc.tile_pool(name="prod", bufs=1))
            zppool = _es.enter_context(tc.tile_pool(name="zp", bufs=1))
            dpool = _es.enter_context(tc.tile_pool(name="dstk", bufs=4))
            spool = _es.enter_context(tc.tile_pool(name="small", bufs=2))
            apool = _es.enter_context(tc.tile_pool(name="abuf", bufs=2))
            xpool = _es.enter_context(tc.tile_pool(name="xin", bufs=6))
            epool = _es.enter_context(tc.tile_pool(name="ebuf", bufs=3))
            enpool = _es.enter_context(tc.tile_pool(name="enorm", bufs=3))
            opool = _es.enter_context(tc.tile_pool(name="obuf", bufs=3))
            ptp = _es.enter_context(tc.tile_pool(name="pt", bufs=1, space="PSUM"))
            pap = _es.enter_context(tc.tile_pool(name="pacc", bufs=1, space="PSUM"))
            psp = _es.enter_context(tc.tile_pool(name="psmall", bufs=1, space="PSUM"))
            plp = _es.enter_context(tc.tile_pool(name="plog", bufs=3, space="PSUM"))
            pdp = plp
            pyp = _es.enter_context(tc.tile_pool(name="py", bufs=2, space="PSUM"))

            # ---- constants ----
            wc_t = cpool.tile([MID, C], F32R, tag="wc")
            nc.sync.dma_start(wc_t[:], wc_d[:])
            waT_c = []
            wbT_c = []
            for h in range(2):
                wat = cpool.tile([128, MID], BF, tag=f"waT{h}")
                nc.sync.dma_start(wat[:], waT_d[128 * h : 128 * (h + 1), :])
                waT_c.append(wat)
                wbt = cpool.tile([128, MID], BF, tag=f"wbT{h}")
                nc.sync.dma_start(wbt[:], wbT_d[128 * h : 128 * (h + 1), :])
                wbT_c.append(wbt)
            wdnT_t = cpool.tile([MID, C], F32R, tag="wdnT")
            nc.sync.dma_start(wdnT_t[:], wdnT_d[:])
            wdlT_t = cpool.tile([MID, C], F32R, tag="wdlT")
            nc.sync.dma_start(wdlT_t[:], wdlT_d[:])
            wattr_t = cpool.tile([128, C], BF, tag="wattr")
            nc.sync.dma_start(wattr_t[:], wattr_d[:])
            onesn_t = cpool.tile([128, 128], BF, tag="onesn")
            nc.sync.dma_start(onesn_t[:], onesn_d[:])
            onesl_t = cpool.tile([64, 64], BF, tag="onesl")
            nc.sync.dma_start(onesl_t[:], onesl_d[:])
            i2_t = cpool.tile([128, 64], BF, tag="i2")
            nc.sync.dma_start(i2_t[:], i2_d[:])
            eyef_t = cpool.tile([128, 128], F32, tag="eyef")
            nc.sync.dma_start(eyef_t[:], eyef_d[:])
            eyeb_t = cpool.tile([128, 128], BF, tag="eyeb")
            nc.sync.dma_start(eyeb_t[:], eyeb_d[:])
            # [256,1] does not fit 128 partitions; load as two chunks
            betap0 = cpool.tile([128, 1], F32, tag="betap0")
            nc.sync.dma_start(betap0[:], betap_d[0:128, :])
            betap1 = cpool.tile([128, 1], F32, tag="betap1")
            nc.sync.dma_start(betap1[:], betap_d[128:256, :])
            betaps = [betap0, betap1]

            for b in range(NB):
                # ---- load curves (cast to bf16) and xbar-transpose ----
                cnat = []
                for cc in range(2):
                    cn = cnpool.tile([128, N], BF, tag="cnat")
                    nc.gpsimd.dma_start(
                        cn[:], cu_d[b, 128 * cc : 128 * (cc + 1), :]
                    )
                    cnat.append(cn)
                cT = ctpool.tile([128, NT, 256], BF, tag="ctrans")
                for cc in range(2):
                    nc.scalar.dma_start(
                        cT[:, :, 128 * cc : 128 * (cc + 1)],
                        cnat[cc][:],
                        transpose=True,
                    )

                # ---- att via DVE mult + segmented reduce ----
                prod = prpool.tile([128, NT, 256], BF, tag="prod")
                wattr_b = wattr_t[:].rearrange("p (o c) -> p o c", o=1).broadcast_to(
                    [128, NT, 256]
                )
                nc.vector.tensor_tensor(prod[:], cT[:], wattr_b, op=ALU.mult)
                attm = spool.tile([128, NT], F32, tag="attm")
                nc.vector.reduce_sum(attm[:], prod[:], axis=AX.X)
                # attm[q, t] = att[nl = 128 t + q];  n = 2t + (q>=64), l = q%64

                # ---- softmax over l (rows of att) ----
                amT_ps = ptp.tile([NT, 128], F32, tag="tps")
                nc.tensor.transpose(amT_ps[:], attm[:], eyef_t[:])
                amT = spool.tile([NT, 128], F32, tag="amT")
                nc.vector.tensor_copy(amT[:], amT_ps[:])
                # amT[t, j*64 + l] = att[n = 2t + j, l]
                rmax = spool.tile([NT, 2], F32, tag="rmax")
                nc.vector.reduce_max(
                    rmax[:], amT[:].rearrange("t (j l) -> t j l", j=2), axis=AX.X
                )
                nmax = spool.tile([NT, 2], F32, tag="nmax")
                nc.vector.tensor_scalar_mul(nmax[:], rmax[:], -1.0)
                el = spool.tile([NT, 128], F32, tag="el")
                for j in range(2):
                    nc.scalar.activation(
                        el[:, 64 * j : 64 * (j + 1)],
                        amT[:, 64 * j : 64 * (j + 1)],
                        AF.Exp,
                        bias=nmax[:, j : j + 1],
                        scale=1.0,
                    )
                ssum = spool.tile([NT, 2], F32, tag="ssum")
                nc.vector.reduce_sum(
                    ssum[:], el[:].rearrange("t (j l) -> t j l", j=2), axis=AX.X
                )
                rsum = spool.tile([NT, 2], F32, tag="rsum")
                nc.vector.reciprocal(rsum[:], ssum[:])
                slm = spool.tile([NT, 128], BF, tag="slm")
                for j in range(2):
                    nc.vector.tensor_scalar_mul(
                        slm[:, 64 * j : 64 * (j + 1)],
                        el[:, 64 * j : 64 * (j + 1)],
                        rsum[:, j : j + 1],
                    )
                # S_l in mix layout [t, (j, l)] -> transpose to [(j,l), t]
                slT_ps = ptp.tile([128, NT], BF, tag="tps")
                nc.tensor.transpose(slT_ps[:], slm[:], eyeb_t[0:NT, 0:NT])
                slT = spool.tile([128, NT], BF, tag="slT")
                nc.vector.tensor_copy(slT[:], slT_ps[:])

                # ---- softmax over n (across q-halves and t) ----
                m1 = spool.tile([128, 1], F32, tag="m1")
                nc.vector.reduce_max(m1[:], attm[:], axis=AX.X)
                m1u = spool.tile([64, 1], F32, tag="m1u")
                nc.vector.tensor_copy(m1u[:], m1[64:128, :])
                mc = spool.tile([64, 1], F32, tag="mc")
                nc.vector.tensor_tensor(mc[:], m1[0:64, :], m1u[:], op=ALU.max)
                nmc = spool.tile([64, 1], F32, tag="nmc")
                nc.vector.tensor_scalar_mul(nmc[:], mc[:], -1.0)
                nmf = spool.tile([128, 1], F32, tag="nmf")
                nc.vector.tensor_copy(nmf[0:64, :], nmc[:])
                nc.vector.tensor_copy(nmf[64:128, :], nmc[:])
                en = spool.tile([128, NT], F32, tag="en")
                nc.scalar.activation(
                    en[:], attm[:], AF.Exp, bias=nmf[:], scale=1.0
                )
                s1 = spool.tile([128, 1], F32, tag="s1")
                nc.vector.reduce_sum(s1[:], en[:], axis=AX.X)
                s1u = spool.tile([64, 1], F32, tag="s1u")
                nc.vector.tensor_copy(s1u[:], s1[64:128, :])
                sc = spool.tile([64, 1], F32, tag="sc")
                nc.vector.tensor_tensor(sc[:], s1[0:64, :], s1u[:], op=ALU.add)
                rc = spool.tile([64, 1], F32, tag="rc")
                nc.vector.reciprocal(rc[:], sc[:])
                rf = spool.tile([128, 1], F32, tag="rf")
                nc.vector.tensor_copy(rf[0:64, :], rc[:])
                nc.vector.tensor_copy(rf[64:128, :], rc[:])
                snm = spool.tile([128, NT], F32, tag="snm")
                nc.vector.tensor_scalar_mul(snm[:], en[:], rf[:])
                # snm[q, t] = S_n[n = 2t + (q>=64), l = q%64]

                # ---- build ZP (block-diag S_l weights) ----
                # zp[:, t, 0:128] is the stationary operand for tile t:
                # col (2t+j) within the tile sits at zp[:, t, j]; value
                # S_l[2t+j, l] at partition j*64 + l.  Extra 2 cols of pad
                # per tile make the per-partition pitch 130 so the strided
                # copies below are expressible as a plain 3D AP.
                zp = zppool.tile([128, NT, 130], BF, tag="zp")
                nc.gpsimd.memset(zp[:], 0.0)
                nc.vector.tensor_copy(
                    zp[0:64, :, 0:1],
                    slT[0:64, :].rearrange("p (t o) -> p t o", o=1),
                )
                nc.vector.tensor_copy(
                    zp[64:128, :, 1:2],
                    slT[64:128, :].rearrange("p (t o) -> p t o", o=1),
                )

                # ---- curver_inter^T via 64 accumulated matmuls ----
                # Stationary slice for tile t starts at flat column 128*t of
                # the pitch-130 buffer, so the value for n=2t+j (written at
                # flat col 130*t + j) lands at within-slice column 2t+j --
                # exactly the global output row it must accumulate into.
                zpf = zp[:].rearrange("p t u -> p (t u)")
                ci_ps = pap.tile([128, 256], F32, tag="acc")
                for t in range(NT):
                    nc.tensor.matmul(
                        ci_ps[:],
                        zpf[:, 128 * t : 128 * (t + 1)],
                        cT[:, t, :],
                        start=(t == 0),
                        stop=(t == NT - 1),
                    )
                ciT = spool.tile([128, 256], BF, tag="ciT")
                nc.vector.tensor_copy(ciT[:], ci_ps[:])

                # ---- curves_intra^T via 64 accumulated matmuls ----
                ct_ps = pap.tile([64, 256], F32, tag="acc")
                for t in range(NT):
                    dt = dpool.tile([128, 64], BF, tag="dstk")
                    nc.vector.tensor_scalar_mul(dt[:], i2_t[:], snm[:, t : t + 1])
                    nc.tensor.matmul(
                        ct_ps[:],
                        dt[:],
                        cT[:, t, :],
                        start=(t == 0),
                        stop=(t == NT - 1),
                    )
                ctT = spool.tile([64, 256], BF, tag="ctT")
                nc.vector.tensor_copy(ctT[:], ct_ps[:])

                # ---- a = wa @ curver_inter ----
                a_ps = psp.tile([128, 128], F32, tag="sm")
                for h in range(2):
                    tr_ps = ptp.tile([128, 128], BF, tag="tps")
                    nc.tensor.transpose(
                        tr_ps[:], ciT[:, 128 * h : 128 * (h + 1)], eyeb_t[:]
                    )
                    cin = spool.tile([128, 128], BF, tag="cin")
                    nc.vector.tensor_copy(cin[:], tr_ps[:])
                    nc.tensor.matmul(
                        a_ps[:],
                        waT_c[h][:],
                        cin[:],
                        start=(h == 0),
                        stop=(h == 1),
                    )
                a_sb = apool.tile([MID, 128], F32R, tag="a_sb")
                nc.vector.tensor_copy(a_sb[:], a_ps[:])

                # ---- b = wb @ curves_intra ----
                b_ps = psp.tile([128, 64], F32, tag="sm")
                for h in range(2):
                    tr2_ps = ptp.tile([128, 64], BF, tag="tps")
                    nc.tensor.transpose(
                        tr2_ps[:], ctT[:, 128 * h : 128 * (h + 1)], eyeb_t[0:64, 0:64]
                    )
                    ctn = spool.tile([128, 64], BF, tag="ctn")
                    nc.vector.tensor_copy(ctn[:], tr2_ps[:])
                    nc.tensor.matmul(
                        b_ps[:],
                        wbT_c[h][:],
                        ctn[:],
                        start=(h == 0),
                        stop=(h == 1),
                    )
                b_sb = apool.tile([MID, 64], F32R, tag="b_sb")
                nc.vector.tensor_copy(b_sb[:], b_ps[:])

                # ---- AiT / AtT (wc.T @ a, wc.T @ b) ----
                AiT = []
                AtT = []
                for cc in range(2):
                    ai_ps = psp.tile([128, 128], F32, tag="sm")
                    nc.tensor.matmul(
                        ai_ps[:],
                        _r(wc_t[:, 128 * cc : 128 * (cc + 1)]),
                        _r(a_sb[:]),
                        start=True,
                        stop=True,
                    )
                    ai = apool.tile([128, 128], F32R, tag="ai")
                    nc.vector.tensor_copy(ai[:], ai_ps[:])
                    AiT.append(ai)
                    at_ps = psp.tile([128, 64], F32, tag="sm")
                    nc.tensor.matmul(
                        at_ps[:],
                        _r(wc_t[:, 128 * cc : 128 * (cc + 1)]),
                        _r(b_sb[:]),
                        start=True,
                        stop=True,
                    )
                    at = apool.tile([128, 64], F32R, tag="at")
                    nc.vector.tensor_copy(at[:], at_ps[:])
                    AtT.append(at)

                # ---- UiT / UtT ----
                ui_ps = psp.tile([128, 256], F32, tag="sm")
                nc.tensor.matmul(
                    ui_ps[:], _r(a_sb[:]), _r(wdnT_t[:]), start=True, stop=True
                )
                UiT = apool.tile([128, 256], BF, tag="UiT")
                nc.vector.tensor_copy(UiT[:], ui_ps[:])
                ut_ps = psp.tile([64, 256], F32, tag="sm")
                nc.tensor.matmul(
                    ut_ps[:], _r(b_sb[:]), _r(wdlT_t[:]), start=True, stop=True
                )
                UtT = apool.tile([64, 256], BF, tag="UtT")
                nc.vector.tensor_copy(UtT[:], ut_ps[:])

                # ---- points loop ----
                for k in range(NCH):
                    ks = CHW * k
                    xc = []
                    for cc in range(2):
                        xt = xpool.tile([128, CHW], F32R, tag="xin")
                        nc.sync.dma_start(
                            xt[:], x_d[b, 128 * cc : 128 * (cc + 1), ks : ks + CHW]
                        )
                        xc.append(xt)

                    pi = plp.tile([128, CHW], F32, tag="pl")
                    pt = plp.tile([64, CHW], F32, tag="pl")
                    for cc in range(2):
                        nc.tensor.matmul(
                            pi[:], _r(AiT[cc][:]), _r(xc[cc][:]),
                            start=(cc == 0), stop=(cc == 1),
                        )
                    for cc in range(2):
                        nc.tensor.matmul(
                            pt[:], _r(AtT[cc][:]), _r(xc[cc][:]),
                            start=(cc == 0), stop=(cc == 1),
                        )
                    ei = epool.tile([128, CHW], BF, tag="ei")
                    nc.scalar.activation(ei[:], pi[:], AF.Exp)
                    et = epool.tile([64, CHW], BF, tag="et")
                    nc.scalar.activation(et[:], pt[:], AF.Exp)

                    di = pdp.tile([128, CHW], F32, tag="pl")
                    nc.tensor.matmul(
                        di[:], onesn_t[:], ei[:], start=True, stop=True
                    )
                    dt_ = pdp.tile([64, CHW], F32, tag="pl")
                    nc.tensor.matmul(
                        dt_[:], onesl_t[:], et[:], start=True, stop=True
                    )
                    ri = enpool.tile([128, CHW], BF, tag="ri")
                    _act_recip(nc, ri[:], di[:])
                    rt = enpool.tile([64, CHW], BF, tag="rt")
                    _act_recip(nc, rt[:], dt_[:])
                    ein = enpool.tile([128, CHW], BF, tag="ein")
                    nc.vector.tensor_tensor(ein[:], ei[:], ri[:], op=ALU.mult)
                    etn = enpool.tile([64, CHW], BF, tag="etn")
                    nc.vector.tensor_tensor(etn[:], et[:], rt[:], op=ALU.mult)

                    for cc in range(2):
                        yps = pyp.tile([128, CHW], F32, tag="yps")
                        nc.tensor.matmul(
                            yps[:],
                            UiT[:, 128 * cc : 128 * (cc + 1)],
                            ein[:],
                            start=True,
                            stop=False,
                        )
                        nc.tensor.matmul(
                            yps[:],
                            UtT[:, 128 * cc : 128 * (cc + 1)],
                            etn[:],
                            start=False,
                            stop=True,
                        )
                        t1 = opool.tile([128, CHW], F32, tag="t1")
                        nc.vector.scalar_tensor_tensor(
                            t1[:], yps[:], betaps[cc][:], xc[cc][:].bitcast(F32),
                            op0=ALU.add, op1=ALU.add,
                        )
                        osb = opool.tile([128, CHW], F32, tag="osb")
                        nc.vector.scalar_tensor_tensor(
                            osb[:], t1[:], 0.2, t1[:],
                            op0=ALU.mult, op1=ALU.max,
                        )
                        nc.sync.dma_start(
                            y_d[b, 128 * cc : 128 * (cc + 1), ks : ks + CHW],
                            osb[:],
                        )
    _split_excess_waits(nc, max_waits=1)
    return nc


_CACHE = {}


def _get_bass():
    if "nc" not in _CACHE:
        _CACHE["nc"] = _build_bass()
    return _CACHE["nc"]


def prepare(x, curves, w_att, wa, wb, wc, wn, wl, wd,
            bn_gamma, bn_beta, bn_mean, bn_var):
    """Build (nc, in_maps) for the SPMD run — shared by kernel() and bench."""
    x = np.ascontiguousarray(np.asarray(x, dtype=np.float32))
    curves = np.ascontiguousarray(np.asarray(curves, dtype=np.float32))
    w_att = np.asarray(w_att, dtype=np.float32)
    wa = np.asarray(wa, dtype=np.float32)
    wb = np.asarray(wb, dtype=np.float32)
    wc = np.asarray(wc, dtype=np.float32)
    wn = np.asarray(wn, dtype=np.float32)
    wl = np.asarray(wl, dtype=np.float32)
    wd = np.asarray(wd, dtype=np.float32)
    bn_gamma = np.asarray(bn_gamma, dtype=np.float32)
    bn_beta = np.asarray(bn_beta, dtype=np.float32)
    bn_mean = np.asarray(bn_mean, dtype=np.float32)
    bn_var = np.asarray(bn_var, dtype=np.float32)

    s = bn_gamma / np.sqrt(bn_var + BN_EPS)
    betap = (bn_beta - bn_mean * s).astype(np.float32).reshape(C, 1)
    wdnT = ((wd[:, :MID] @ wn).T * s[None, :]).astype(np.float32)
    wdlT = ((wd[:, MID:] @ wl).T * s[None, :]).astype(np.float32)

    consts = {
        "wc": np.ascontiguousarray(wc),
        "waT": np.ascontiguousarray(wa.T).astype(bf16),
        "wbT": np.ascontiguousarray(wb.T).astype(bf16),
        "wdnT": np.ascontiguousarray(wdnT),
        "wdlT": np.ascontiguousarray(wdlT),
        "wattr": np.ascontiguousarray(
            np.broadcast_to(w_att.reshape(1, C), (128, C))
        ).astype(bf16),
        "onesn": np.ones((128, 128), bf16),
        "onesl": np.ones((64, 64), bf16),
        "i2": np.concatenate([np.eye(64), np.eye(64)], axis=0).astype(bf16),
        "eyef": np.eye(128, dtype=np.float32),
        "eyeb": np.eye(128).astype(bf16),
        "betap": betap,
    }

    curves_flat = curves.reshape(B, C, CN * CL)
    in_maps = []
    for core in range(NCORES):
        b0 = core * NB
        m = dict(consts)
        m["x"] = np.ascontiguousarray(x[b0 : b0 + NB])
        m["curves"] = np.ascontiguousarray(curves_flat[b0 : b0 + NB])
        in_maps.append(m)

    nc = _get_bass()
    return nc, in_maps


def kernel(x, curves, w_att, wa, wb, wc, wn, wl, wd,
           bn_gamma, bn_beta, bn_mean, bn_var):
    nc, in_maps = prepare(x, curves, w_att, wa, wb, wc, wn, wl, wd,
                          bn_gamma, bn_beta, bn_mean, bn_var)
    res = run_bass_kernel_spmd(nc, in_maps, core_ids=list(range(NCORES)))
    out = np.empty((B, C, N), np.float32)
    for core in range(NCORES):
        out[core * NB : (core + 1) * NB] = res.results[core]["y"]
    return out



# revision 6
# speedup vs baseline: 221.1827x; 221.1827x over previous
"""Trainium2 Bass kernel for nn_CIC (curve-attention GNN message passing block).

Sharding: data-parallel over batch B=16 -> 2 batches per core x 8 cores.
All weights replicated; no collectives.

Math (per batch b):
  att[n,l]   = sum_c w_att[c] * curves[c,n,l]
  S_l        = softmax(att, axis=l);  S_n = softmax(att, axis=n)
  curver_inter[c,n] = sum_l curves[c,n,l] * S_l[n,l]
  curves_intra[c,l] = sum_n curves[c,n,l] * S_n[n,l]
  a = wa @ curver_inter              [MID, n]
  b = wb @ curves_intra              [MID, l]
  AiT = wc.T @ a   [C, n];  AtT = wc.T @ b  [C, l]      (folded x_logits)
  logits_i = AiT.T @ x  [n, Np];  logits_t = AtT.T @ x  [l, Np]
  E = exp(logits); den = colsum(E)  (ones-matmul, replicated rows)
  UiT = a.T @ (wd[:, :MID] @ wn).T * s   [n, C]   (BN scale s folded)
  UtT = b.T @ (wd[:, MID:] @ wl).T * s   [l, C]
  y[c,p] = sum_n UiT[n,c] * E_i_norm[n,p] + sum_l UtT[l,c] * E_t_norm[l,p]
  out = leaky_relu(x + y + (beta - mean*s), 0.2)

Layout tricks:
  - curves cast to bf16 during the HBM load (SWDGE cast dma), then
    xbar-DMA-transposed to curvesT[p, t, c] = curves_bf16[c, 128*t + p].
  - curver_inter/curves_intra via 64 accumulated matmuls with sparse
    stationary weights built from S_l / S_n (block-diag / dual-diag).
  - att via one big DVE multiply + segmented reduce over curvesT.
  - Big matmul streams run as float32r (full PE rate at free dim >= 256).
"""

import numpy as np
import ml_dtypes

import concourse.bass as bass
import concourse.mybir as mybir
from concourse.tile import TileContext
from concourse.bass_utils import run_bass_kernel_spmd

bf16 = ml_dtypes.bfloat16
F32 = mybir.dt.float32
BF = mybir.dt.bfloat16
F32R = mybir.dt.float32r
ALU = mybir.AluOpType
AF = mybir.ActivationFunctionType
AX = mybir.AxisListType

B, C, N = 16, 256, 8192
CN, CL, MID = 128, 64, 128
NCORES = 8
NB = B // NCORES          # batches per core
NT = CN // 2              # 64 nl-tiles of 128 per batch
CHW = 512                 # points chunk width
NCH = N // CHW            # 16 chunks
BN_EPS = 1e-5


_WS_COUNTER = [0]


def _split_excess_waits(nc, max_waits=1):
    """This walrus build rejects instructions carrying more than ~1 sem-wait
    command.  Move excess waits onto same-engine NoOps inserted right before
    the offending instruction (program order on one engine preserves the
    semantics exactly)."""
    for fn in nc.m.functions:
        for blk in fn.blocks:
            insts = list(blk.instructions)
            out = []
            changed = False
            for inst in insts:
                si = inst.sync_info
                waits = list(si.on_wait) if si and si.on_wait else []
                if len(waits) > max_waits and inst.engine is not None:
                    keep = waits[:max_waits]
                    extra = waits[max_waits:]
                    for w in extra:
                        _WS_COUNTER[0] += 1
                        nop = mybir.InstNoOp(
                            name=f"I-waitsplit-{_WS_COUNTER[0]}",
                            opcode="NoOp",
                            engine=inst.engine,
                            ins=[],
                            outs=[],
                            sync_info=mybir.SyncInfo(on_wait=[w], on_update=[]),
                        )
                        out.append(nop)
                    si.on_wait = keep
                    changed = True
                out.append(inst)
            if changed:
                blk.instructions = out


def _act_recip(nc, out_ap, in_ap):
    """ACT-engine reciprocal via raw InstActivation (the bass wrapper refuses
    Reciprocal for accuracy reasons; softmax denominators tolerate it)."""
    eng = nc.scalar
    imm = lambda v: mybir.ImmediateValue(dtype=mybir.dt.float32, value=v)
    return eng.add_instruction(
        mybir.InstActivation(
            name=nc.get_next_instruction_name(),
            func=AF.Reciprocal,
            ins=[eng.lower_ap(in_ap), imm(0.0), imm(1.0), imm(0.0)],
            outs=[eng.lower_ap(out_ap)],
        )
    )


def _r(ap):
    """View an fp32 AP as float32r for full-rate PE streaming."""
    return ap.bitcast(F32R)


def _build_bass(n_reps=1):
    nc = bass.Bass()

    x_d = nc.dram_tensor("x", [NB, C, N], F32R, kind="ExternalInput")
    cu_d = nc.dram_tensor("curves", [NB, C, N], F32, kind="ExternalInput")
    wc_d = nc.dram_tensor("wc", [MID, C], F32R, kind="ExternalInput")
    waT_d = nc.dram_tensor("waT", [C, MID], BF, kind="ExternalInput")
    wbT_d = nc.dram_tensor("wbT", [C, MID], BF, kind="ExternalInput")
    wdnT_d = nc.dram_tensor("wdnT", [MID, C], F32R, kind="ExternalInput")
    wdlT_d = nc.dram_tensor("wdlT", [MID, C], F32R, kind="ExternalInput")
    wattr_d = nc.dram_tensor("wattr", [128, C], BF, kind="ExternalInput")
    onesn_d = nc.dram_tensor("onesn", [128, 128], BF, kind="ExternalInput")
    onesl_d = nc.dram_tensor("onesl", [64, 64], BF, kind="ExternalInput")
    i2_d = nc.dram_tensor("i2", [128, 64], BF, kind="ExternalInput")
    eyef_d = nc.dram_tensor("eyef", [128, 128], F32, kind="ExternalInput")
    eyeb_d = nc.dram_tensor("eyeb", [128, 128], BF, kind="ExternalInput")
    betap_d = nc.dram_tensor("betap", [C, 1], F32, kind="ExternalInput")
    y_d = nc.dram_tensor("y", [NB, C, N], F32, kind="ExternalOutput")

    from contextlib import ExitStack

    with TileContext(nc) as tc:
        with ExitStack() as _es:
            cpool = _es.enter_context(tc.tile_pool(name="const", bufs=1))
            cnpool = _es.enter_context(tc.tile_pool(name="cnat", bufs=2))
            ctpool = _es.enter_context(tc.tile_pool(name="ctrans", bufs=1))
            prpool = _es.enter_context(tc.tile_pool(name="prod", bufs=1))
            zppool = _es.enter_context(tc.tile_pool(name="zp", bufs=1))
            dpool = _es.enter_context(tc.tile_pool(name="dstk", bufs=4))
            spool = _es.enter_context(tc.tile_pool(name="small", bufs=2))
            apool = _es.enter_context(tc.tile_pool(name="abuf", bufs=2))
            xpool = _es.enter_context(tc.tile_pool(name="xin", bufs=6))
            epool = _es.enter_context(tc.tile_pool(name="ebuf", bufs=3))
            enpool = _es.enter_context(tc.tile_pool(name="enorm", bufs=3))
            opool = _es.enter_context(tc.tile_pool(name="obuf", bufs=3))
            ptp = _es.enter_context(tc.tile_pool(name="pt", bufs=1, space="PSUM"))
            pap = _es.enter_context(tc.tile_pool(name="pacc", bufs=1, space="PSUM"))
            psp = _es.enter_context(tc.tile_pool(name="psmall", bufs=1, space="PSUM"))
            plp = _es.enter_context(tc.tile_pool(name="plog", bufs=3, space="PSUM"))
            pdp = plp
            pyp = _es.enter_context(tc.tile_pool(name="py", bufs=2, space="PSUM"))

            # ---- constants ----
            wc_t = cpool.tile([MID, C], F32R, tag="wc")
            nc.sync.dma_start(wc_t[:], wc_d[:])
            waT_c = []
            wbT_c = []
            for h in range(2):
                wat = cpool.tile([128, MID], BF, tag=f"waT{h}")
                nc.sync.dma_start(wat[:], waT_d[128 * h : 128 * (h + 1), :])
                waT_c.append(wat)
                wbt = cpool.tile([128, MID], BF, tag=f"wbT{h}")
                nc.sync.dma_start(wbt[:], wbT_d[128 * h : 128 * (h + 1), :])
                wbT_c.append(wbt)
            wdnT_t = cpool.tile([MID, C], F32R, tag="wdnT")
            nc.sync.dma_start(wdnT_t[:], wdnT_d[:])
            wdlT_t = cpool.tile([MID, C], F32R, tag="wdlT")
            nc.sync.dma_start(wdlT_t[:], wdlT_d[:])
            wattr_t = cpool.tile([128, C], BF, tag="wattr")
            nc.sync.dma_start(wattr_t[:], wattr_d[:])
            onesn_t = cpool.tile([128, 128], BF, tag="onesn")
            nc.sync.dma_start(onesn_t[:], onesn_d[:])
            onesl_t = cpool.tile([64, 64], BF, tag="onesl")
            nc.sync.dma_start(onesl_t[:], onesl_d[:])
            i2_t = cpool.tile([128, 64], BF, tag="i2")
            nc.sync.dma_start(i2_t[:], i2_d[:])
            eyef_t = cpool.tile([128, 128], F32, tag="eyef")
            nc.sync.dma_start(eyef_t[:], eyef_d[:])
            eyeb_t = cpool.tile([128, 128], BF, tag="eyeb")
            nc.sync.dma_start(eyeb_t[:], eyeb_d[:])
            # [256,1] does not fit 128 partitions; load as two chunks
            betap0 = cpool.tile([128, 1], F32, tag="betap0")
            nc.sync.dma_start(betap0[:], betap_d[0:128, :])
            betap1 = cpool.tile([128, 1], F32, tag="betap1")
            nc.sync.dma_start(betap1[:], betap_d[128:256, :])
            betaps = [betap0, betap1]

            for b in [b for _ in range(n_reps) for b in range(NB)]:
                # ---- load curves (cast to bf16) and xbar-transpose ----
                cnat = []
                for cc in range(2):
                    cn = cnpool.tile([128, N], BF, tag="cnat")
                    nc.gpsimd.dma_start(
                        cn[:], cu_d[b, 128 * cc : 128 * (cc + 1), :]
                    )
                    cnat.append(cn)
                cT = ctpool.tile([128, NT, 256], BF, tag="ctrans")
                for cc in range(2):
                    nc.scalar.dma_start(
                        cT[:, :, 128 * cc : 128 * (cc + 1)],
                        cnat[cc][:],
                        transpose=True,
                    )

                # ---- att via DVE mult + segmented reduce ----
                prod = prpool.tile([128, NT, 256], BF, tag="prod")
                wattr_b = wattr_t[:].rearrange("p (o c) -> p o c", o=1).broadcast_to(
                    [128, NT, 256]
                )
                nc.vector.tensor_tensor(prod[:], cT[:], wattr_b, op=ALU.mult)
                attm = spool.tile([128, NT], F32, tag="attm")
                nc.vector.reduce_sum(attm[:], prod[:], axis=AX.X)
                # attm[q, t] = att[nl = 128 t + q];  n = 2t + (q>=64), l = q%64

                # ---- softmax over l (rows of att) ----
                amT_ps = ptp.tile([NT, 128], F32, tag="tps")
                nc.tensor.transpose(amT_ps[:], attm[:], eyef_t[:])
                amT = spool.tile([NT, 128], F32, tag="amT")
                nc.vector.tensor_copy(amT[:], amT_ps[:])
                # amT[t, j*64 + l] = att[n = 2t + j, l]
                rmax = spool.tile([NT, 2], F32, tag="rmax")
                nc.vector.reduce_max(
                    rmax[:], amT[:].rearrange("t (j l) -> t j l", j=2), axis=AX.X
                )
                nmax = spool.tile([NT, 2], F32, tag="nmax")
                nc.vector.tensor_scalar_mul(nmax[:], rmax[:], -1.0)
                el = spool.tile([NT, 128], F32, tag="el")
                for j in range(2):
                    nc.scalar.activation(
                        el[:, 64 * j : 64 * (j + 1)],
                        amT[:, 64 * j : 64 * (j + 1)],
                        AF.Exp,
                        bias=nmax[:, j : j + 1],
                        scale=1.0,
                    )
                ssum = spool.tile([NT, 2], F32, tag="ssum")
                nc.vector.reduce_sum(
                    ssum[:], el[:].rearrange("t (j l) -> t j l", j=2), axis=AX.X
                )
                rsum = spool.tile([NT, 2], F32, tag="rsum")
                nc.vector.reciprocal(rsum[:], ssum[:])
                slm = spool.tile([NT, 128], BF, tag="slm")
                for j in range(2):
                    nc.vector.tensor_scalar_mul(
                        slm[:, 64 * j : 64 * (j + 1)],
                        el[:, 64 * j : 64 * (j + 1)],
                        rsum[:, j : j + 1],
                    )
                # S_l in mix layout [t, (j, l)] -> transpose to [(j,l), t]
                slT_ps = ptp.tile([128, NT], BF, tag="tps")
                nc.tensor.transpose(slT_ps[:], slm[:], eyeb_t[0:NT, 0:NT])
                slT = spool.tile([128, NT], BF, tag="slT")
                nc.vector.tensor_copy(slT[:], slT_ps[:])

                # ---- softmax over n (across q-halves and t) ----
                m1 = spool.tile([128, 1], F32, tag="m1")
                nc.vector.reduce_max(m1[:], attm[:], axis=AX.X)
                m1u = spool.tile([64, 1], F32, tag="m1u")
                nc.vector.tensor_copy(m1u[:], m1[64:128, :])
                mc = spool.tile([64, 1], F32, tag="mc")
                nc.vector.tensor_tensor(mc[:], m1[0:64, :], m1u[:], op=ALU.max)
                nmc = spool.tile([64, 1], F32, tag="nmc")
                nc.vector.tensor_scalar_mul(nmc[:], mc[:], -1.0)
                nmf = spool.tile([128, 1], F32, tag="nmf")
                nc.vector.tensor_copy(nmf[0:64, :], nmc[:])
                nc.vector.tensor_copy(nmf[64:128, :], nmc[:])
                en = spool.tile([128, NT], F32, tag="en")
                nc.scalar.activation(
                    en[:], attm[:], AF.Exp, bias=nmf[:], scale=1.0
                )
                s1 = spool.tile([128, 1], F32, tag="s1")
                nc.vector.reduce_sum(s1[:], en[:], axis=AX.X)
                s1u = spool.tile([64, 1], F32, tag="s1u")
                nc.vector.tensor_copy(s1u[:], s1[64:128, :])
                sc = spool.tile([64, 1], F32, tag="sc")
                nc.vector.tensor_tensor(sc[:], s1[0:64, :], s1u[:], op=ALU.add)
                rc = spool.tile([64, 1], F32, tag="rc")
                nc.vector.reciprocal(rc[:], sc[:])
                rf = spool.tile([128, 1], F32, tag="rf")
                nc.vector.tensor_copy(rf[0:64, :], rc[:])
                nc.vector.tensor_copy(rf[64:128, :], rc[:])
                snm = spool.tile([128, NT], F32, tag="snm")
                nc.vector.tensor_scalar_mul(snm[:], en[:], rf[:])
                # snm[q, t] = S_n[n = 2t + (q>=64), l = q%64]

                # ---- build ZP (block-diag S_l weights) ----
                # zp[:, t, 0:128] is the stationary operand for tile t:
                # col (2t+j) within the tile sits at zp[:, t, j]; value
                # S_l[2t+j, l] at partition j*64 + l.  Extra 2 cols of pad
                # per tile make the per-partition pitch 130 so the strided
                # copies below are expressible as a plain 3D AP.
                zp = zppool.tile([128, NT, 130], BF, tag="zp")
                nc.gpsimd.memset(zp[:], 0.0)
                nc.vector.tensor_copy(
                    zp[0:64, :, 0:1],
                    slT[0:64, :].rearrange("p (t o) -> p t o", o=1),
                )
                nc.vector.tensor_copy(
                    zp[64:128, :, 1:2],
                    slT[64:128, :].rearrange("p (t o) -> p t o", o=1),
                )

                # ---- curver_inter^T via 64 accumulated matmuls ----
                # Stationary slice for tile t starts at flat column 128*t of
                # the pitch-130 buffer, so the value for n=2t+j (written at
                # flat col 130*t + j) lands at within-slice column 2t+j --
                # exactly the global output row it must accumulate into.
                zpf = zp[:].rearrange("p t u -> p (t u)")
                ci_ps = pap.tile([128, 256], F32, tag="acc")
                for t in range(NT):
                    nc.tensor.matmul(
                        ci_ps[:],
                        zpf[:, 128 * t : 128 * (t + 1)],
                        cT[:, t, :],
                        start=(t == 0),
                        stop=(t == NT - 1),
                    )
                ciT = spool.tile([128, 256], BF, tag="ciT")
                nc.vector.tensor_copy(ciT[:], ci_ps[:])

                # ---- curves_intra^T via 64 accumulated matmuls ----
                ct_ps = pap.tile([64, 256], F32, tag="acc")
                for t in range(NT):
                    dt = dpool.tile([128, 64], BF, tag="dstk")
                    nc.vector.tensor_scalar_mul(dt[:], i2_t[:], snm[:, t : t + 1])
                    nc.tensor.matmul(
                        ct_ps[:],
                        dt[:],
                        cT[:, t, :],
                        start=(t == 0),
                        stop=(t == NT - 1),
                    )
                ctT = spool.tile([64, 256], BF, tag="ctT")
                nc.vector.tensor_copy(ctT[:], ct_ps[:])

                # ---- a = wa @ curver_inter ----
                a_ps = psp.tile([128, 128], F32, tag="sm")
                for h in range(2):
                    tr_ps = ptp.tile([128, 128], BF, tag="tps")
                    nc.tensor.transpose(
                        tr_ps[:], ciT[:, 128 * h : 128 * (h + 1)], eyeb_t[:]
                    )
                    cin = spool.tile([128, 128], BF, tag="cin")
                    nc.vector.tensor_copy(cin[:], tr_ps[:])
                    nc.tensor.matmul(
                        a_ps[:],
                        waT_c[h][:],
                        cin[:],
                        start=(h == 0),
                        stop=(h == 1),
                    )
                a_sb = apool.tile([MID, 128], F32R, tag="a_sb")
                nc.vector.tensor_copy(a_sb[:], a_ps[:])

                # ---- b = wb @ curves_intra ----
                b_ps = psp.tile([128, 64], F32, tag="sm")
                for h in range(2):
                    tr2_ps = ptp.tile([128, 64], BF, tag="tps")
                    nc.tensor.transpose(
                        tr2_ps[:], ctT[:, 128 * h : 128 * (h + 1)], eyeb_t[0:64, 0:64]
                    )
                    ctn = spool.tile([128, 64], BF, tag="ctn")
                    nc.vector.tensor_copy(ctn[:], tr2_ps[:])
                    nc.tensor.matmul(
                        b_ps[:],
                        wbT_c[h][:],
                        ctn[:],
                        start=(h == 0),
                        stop=(h == 1),
                    )
                b_sb = apool.tile([MID, 64], F32R, tag="b_sb")
                nc.vector.tensor_copy(b_sb[:], b_ps[:])

                # ---- AiT / AtT (wc.T @ a, wc.T @ b) ----
                AiT = []
                AtT = []
                for cc in range(2):
                    ai_ps = psp.tile([128, 128], F32, tag="sm")
                    nc.tensor.matmul(
                        ai_ps[:],
                        _r(wc_t[:, 128 * cc : 128 * (cc + 1)]),
                        _r(a_sb[:]),
                        start=True,
                        stop=True,
                    )
                    ai = apool.tile([128, 128], F32R, tag="ai")
                    nc.vector.tensor_copy(ai[:], ai_ps[:])
                    AiT.append(ai)
                    at_ps = psp.tile([128, 64], F32, tag="sm")
                    nc.tensor.matmul(
                        at_ps[:],
                        _r(wc_t[:, 128 * cc : 128 * (cc + 1)]),
                        _r(b_sb[:]),
                        start=True,
                        stop=True,
                    )
                    at = apool.tile([128, 64], F32R, tag="at")
                    nc.vector.tensor_copy(at[:], at_ps[:])
                    AtT.append(at)

                # ---- UiT / UtT ----
                ui_ps = psp.tile([128, 256], F32, tag="sm")
                nc.tensor.matmul(
                    ui_ps[:], _r(a_sb[:]), _r(wdnT_t[:]), start=True, stop=True
                )
                UiT = apool.tile([128, 256], BF, tag="UiT")
                nc.vector.tensor_copy(UiT[:], ui_ps[:])
                ut_ps = psp.tile([64, 256], F32, tag="sm")
                nc.tensor.matmul(
                    ut_ps[:], _r(b_sb[:]), _r(wdlT_t[:]), start=True, stop=True
                )
                UtT = apool.tile([64, 256], BF, tag="UtT")
                nc.vector.tensor_copy(UtT[:], ut_ps[:])

                # ---- points loop ----
                for k in range(NCH):
                    ks = CHW * k
                    xc = []
                    for cc in range(2):
                        xt = xpool.tile([128, CHW], F32R, tag="xin")
                        nc.sync.dma_start(
                            xt[:], x_d[b, 128 * cc : 128 * (cc + 1), ks : ks + CHW]
                        )
                        xc.append(xt)

                    pi = plp.tile([128, CHW], F32, tag="pl")
                    pt = plp.tile([64, CHW], F32, tag="pl")
                    for cc in range(2):
                        nc.tensor.matmul(
                            pi[:], _r(AiT[cc][:]), _r(xc[cc][:]),
                            start=(cc == 0), stop=(cc == 1),
                        )
                    for cc in range(2):
                        nc.tensor.matmul(
                            pt[:], _r(AtT[cc][:]), _r(xc[cc][:]),
                            start=(cc == 0), stop=(cc == 1),
                        )
                    ei = epool.tile([128, CHW], BF, tag="ei")
                    nc.scalar.activation(ei[:], pi[:], AF.Exp)
                    et = epool.tile([64, CHW], BF, tag="et")
                    nc.scalar.activation(et[:], pt[:], AF.Exp)

                    di = pdp.tile([128, CHW], F32, tag="pl")
                    nc.tensor.matmul(
                        di[:], onesn_t[:], ei[:], start=True, stop=True
                    )
                    dt_ = pdp.tile([64, CHW], F32, tag="pl")
                    nc.tensor.matmul(
                        dt_[:], onesl_t[:], et[:], start=True, stop=True
                    )
                    ri = enpool.tile([128, CHW], BF, tag="ri")
                    _act_recip(nc, ri[:], di[:])
                    rt = enpool.tile([64, CHW], BF, tag="rt")
                    _act_recip(nc, rt[:], dt_[:])
                    ein = enpool.tile([128, CHW], BF, tag="ein")
                    nc.vector.tensor_tensor(ein[:], ei[:], ri[:], op=ALU.mult)
                    etn = enpool.tile([64, CHW], BF, tag="etn")
                    nc.vector.tensor_tensor(etn[:], et[:], rt[:], op=ALU.mult)

                    for cc in range(2):
                        yps = pyp.tile([128, CHW], F32, tag="yps")
                        nc.tensor.matmul(
                            yps[:],
                            UiT[:, 128 * cc : 128 * (cc + 1)],
                            ein[:],
                            start=True,
                            stop=False,
                        )
                        nc.tensor.matmul(
                            yps[:],
                            UtT[:, 128 * cc : 128 * (cc + 1)],
                            etn[:],
                            start=False,
                            stop=True,
                        )
                        t1 = opool.tile([128, CHW], F32, tag="t1")
                        nc.vector.scalar_tensor_tensor(
                            t1[:], yps[:], betaps[cc][:], xc[cc][:].bitcast(F32),
                            op0=ALU.add, op1=ALU.add,
                        )
                        osb = opool.tile([128, CHW], F32, tag="osb")
                        nc.vector.scalar_tensor_tensor(
                            osb[:], t1[:], 0.2, t1[:],
                            op0=ALU.mult, op1=ALU.max,
                        )
                        nc.sync.dma_start(
                            y_d[b, 128 * cc : 128 * (cc + 1), ks : ks + CHW],
                            osb[:],
                        )
    _split_excess_waits(nc, max_waits=1)
    return nc


_CACHE = {}


def _get_bass(n_reps=1):
    key = ("nc", n_reps)
    if key not in _CACHE:
        _CACHE[key] = _build_bass(n_reps)
    return _CACHE[key]


def prepare(x, curves, w_att, wa, wb, wc, wn, wl, wd,
            bn_gamma, bn_beta, bn_mean, bn_var):
    """Build (nc, in_maps) for the SPMD run — shared by kernel() and bench."""
    x = np.ascontiguousarray(np.asarray(x, dtype=np.float32))
    curves = np.ascontiguousarray(np.asarray(curves, dtype=np.float32))
    w_att = np.asarray(w_att, dtype=np.float32)
    wa = np.asarray(wa, dtype=np.float32)
    wb = np.asarray(wb, dtype=np.float32)
    wc = np.asarray(wc, dtype=np.float32)
    wn = np.asarray(wn, dtype=np.float32)
    wl = np.asarray(wl, dtype=np.float32)
    wd = np.asarray(wd, dtype=np.float32)
    bn_gamma = np.asarray(bn_gamma, dtype=np.float32)
    bn_beta = np.asarray(bn_beta, dtype=np.float32)
    bn_mean = np.asarray(bn_mean, dtype=np.float32)
    bn_var = np.asarray(bn_var, dtype=np.float32)

    s = bn_gamma / np.sqrt(bn_var + BN_EPS)
    betap = (bn_beta - bn_mean * s).astype(np.float32).reshape(C, 1)
    wdnT = ((wd[:, :MID] @ wn).T * s[None, :]).astype(np.float32)
    wdlT = ((wd[:, MID:] @ wl).T * s[None, :]).astype(np.float32)

    consts = {
        "wc": np.ascontiguousarray(wc),
        "waT": np.ascontiguousarray(wa.T).astype(bf16),
        "wbT": np.ascontiguousarray(wb.T).astype(bf16),
        "wdnT": np.ascontiguousarray(wdnT),
        "wdlT": np.ascontiguousarray(wdlT),
        "wattr": np.ascontiguousarray(
            np.broadcast_to(w_att.reshape(1, C), (128, C))
        ).astype(bf16),
        "onesn": np.ones((128, 128), bf16),
        "onesl": np.ones((64, 64), bf16),
        "i2": np.concatenate([np.eye(64), np.eye(64)], axis=0).astype(bf16),
        "eyef": np.eye(128, dtype=np.float32),
        "eyeb": np.eye(128).astype(bf16),
        "betap": betap,
    }

    curves_flat = curves.reshape(B, C, CN * CL)
    in_maps = []
    for core in range(NCORES):
        b0 = core * NB
        m = dict(consts)
        m["x"] = np.ascontiguousarray(x[b0 : b0 + NB])
        m["curves"] = np.ascontiguousarray(curves_flat[b0 : b0 + NB])
        in_maps.append(m)

    nc = _get_bass()
    return nc, in_maps


def kernel(x, curves, w_att, wa, wb, wc, wn, wl, wd,
           bn_gamma, bn_beta, bn_mean, bn_var):
    nc, in_maps = prepare(x, curves, w_att, wa, wb, wc, wn, wl, wd,
                          bn_gamma, bn_beta, bn_mean, bn_var)
    res = run_bass_kernel_spmd(nc, in_maps, core_ids=list(range(NCORES)))
    out = np.empty((B, C, N), np.float32)
    for core in range(NCORES):
        out[core * NB : (core + 1) * NB] = res.results[core]["y"]
    return out

